# revision 23
# baseline (speedup 1.0000x reference)
"""Block-sparse top-k masked linear for Trainium2, tensor-parallel over 8 cores.

out = (block_masked x) @ W + bias
  x: (128, 1, 4096) fp16, W: (4096, 11008) fp16, bias: (11008,) fp16
  mask: per (32-row x 64-col) block of x, keep blocks whose mean |x| is
  >= the 32nd-largest of the 64 k-block activations in that row block.

Sharding: column-parallel -- each of the 8 cores gets an 11008/8 = 1376
column slice of W and bias; x is replicated; outputs are concatenated.

Layout/bandwidth strategy (the kernel is HBM-bound at ~12.4MB of reads):
 * W is relaid out on the host to [128, 32*1376] (partition-major: row p
   holds W[kt*128+p, :] for every k-tile kt) so W DMAs move 5.5-11KB of
   contiguous bytes per partition -- big descriptors instead of 2.75KB rows.
 * x is transposed on the host into the same partition-major form, which
   (a) gives the x DMA 8KB lines, (b) deletes the 32 PE transposes + 8
   vector copies of the old on-chip transpose pipeline, and (c) frees the
   vector engine: per-block |x| sums are PE matmuls against 0/1 selector
   matrices (all 32 of which are column-shifted windows of one [128,126]
   constant), accumulated into one PSUM tile.
 * Six DMA rings run in parallel: the two HWDGE rings (sync/scalar), the
   gpsimd SWDGE ring, and SWDGE queues 1-3 driven by identity-index
   dma_gather ops that stream the last 12 k-tiles of W.
"""
from contextlib import ExitStack

import numpy as np

import concourse.bass as bass
import concourse.tile as tile
from concourse import bacc, mybir
from concourse.bass_utils import run_bass_kernel_spmd

F16 = mybir.dt.float16
F32 = mybir.dt.float32
I16 = mybir.dt.int16
AX = mybir.AxisListType
ALU = mybir.AluOpType
ACT = mybir.ActivationFunctionType

M = 128          # rows of x
K = 4096         # contraction
N = 11008        # out features
NCORES = 8
NLOC = N // NCORES           # 1376 columns per core
BLOCK_M, BLOCK_K = 32, 64
NBM, NBK = M // BLOCK_M, K // BLOCK_K   # 4 row blocks, 64 k blocks
KEEP = 32                               # k blocks kept per row block
NKT = K // 128                          # 32 k tiles of 128
N_TILES = [(0, 512), (512, 512), (1024, 352)]   # n-tile offsets/sizes

# direct-ring W chunks: (queue, n_ktiles) in expected arrival order; k-tiles
# are assigned to chunks in this order so the GEMM (program order kt=0..31)
# is never blocked on a late chunk.  gpsimd leads (only 96KB of consts ahead
# of it), sync/scalar follow their 0.33MB x-transpose slices.
_SCHED = [("S", 2), ("A", 2), ("S", 2), ("A", 2)]
NKT_DIRECT = sum(n for _, n in _SCHED)      # 8
GATHER_KT = 8                               # k-tiles per gather (3 gathers)

# blob const columns: [ jh 0:128 | ksel 128:160 | I64 160:224 | ebase 224:350 ]
_EB0 = 224
_EBW = 2 * (NKT - 1) + BLOCK_K              # 126


def _program(ctx: ExitStack, tc: tile.TileContext, ins, outs):
    nc = tc.nc
    xt_d, w_d, wg_d, b_d, blob_d, idx_d = ins
    (o_d,) = outs

    const = ctx.enter_context(tc.tile_pool(name="const", bufs=1))
    mk = ctx.enter_context(tc.tile_pool(name="mk", bufs=1))
    xtp = ctx.enter_context(tc.tile_pool(name="xtp", bufs=1))
    wpool = ctx.enter_context(tc.tile_pool(name="wpool", bufs=len(_SCHED)))
    xmpool = ctx.enter_context(tc.tile_pool(name="xmpool", bufs=8))
    opool = ctx.enter_context(tc.tile_pool(name="opool", bufs=1))
    psum = ctx.enter_context(tc.tile_pool(name="psum", bufs=1, space="PSUM"))

    # consts on the gpsimd ring
    blob = const.tile([128, _EB0 + _EBW], F16)
    nc.gpsimd.dma_start(blob[:], blob_d)
    jh = blob[0:64, 0:128]
    ksel = blob[0:64, 128:128 + NKT]
    i64 = blob[0:64, 160:224]
    bias_sb = const.tile([1, NLOC], F16)
    nc.gpsimd.dma_start(bias_sb[:], b_d)
    # gather idx tables: exact [16, 8] int16 tiles, loaded on the fast
    # sync HWDGE ring so the Q7 desc-gen isn't waiting on its own ring
    idx_ts = []
    for q in range(3):
        it = const.tile([128, 8], I16, name=f"idx{q}", tag="idx")
        nc.sync.dma_start(it[:], idx_d[:, q * 8:(q + 1) * 8])
        idx_ts.append(it)

    # ---- HAM warm-up: ~4us of junk matmuls so the PE clock-gate opens
    # before the GEMM starts (otherwise everything runs at 1.2 GHz)
    warm_sb = mk.tile([128, 512], F16)
    nc.vector.memset(warm_sb[:], 0.0)
    warm_ps = psum.tile([128, 512], F32, name="warm_ps", tag="warm", bufs=1)
    for i in range(9):
        nc.tensor.matmul(warm_ps[:], lhsT=warm_sb[:, 0:128], rhs=warm_sb[:],
                         start=True, stop=True)

    # ---- x (pre-transposed on host): two full-partition column halves.
    # NOTE: never partition-slice a DMA -- a <128-partition transfer uses
    # only a subset of the 16 SDMA engines, so its completion semaphore
    # undercounts and consumers stall until unrelated later DMAs land.
    xt_all = xtp.tile([128, K], F16)
    nc.sync.dma_start(xt_all[:, 0:K // 2], xt_d[:, 0:K // 2],
                      single_packet=True)
    nc.scalar.dma_start(xt_all[:, K // 2:], xt_d[:, K // 2:],
                        single_packet=True)

    # ---- W bulk via three 8-ktile dma_gathers on SWDGE queues 1-3.
    # A gpsimd-side copy of each idx tile forces Q7 desc-gen to run after
    # the idx data has landed (desc-gen reads the idx values directly).
    kt_tile = [None] * NKT    # kt -> (tile, col offset)
    idx_scr = mk.tile([128, 8], I16)
    for q in range(3):
        nc.gpsimd.tensor_copy(idx_scr[:], idx_ts[q][:])
    for q in range(3):
        wgt = wpool.tile([128, GATHER_KT * NLOC], F16, name=f"wg{q}", tag="w")
        nc.gpsimd.dma_gather(
            wgt[:].rearrange("p (c e) -> p c e", c=1),
            wg_d[:, :],
            idx_ts[q][:],
            num_idxs=128, num_idxs_reg=128,
            elem_size=GATHER_KT * NLOC, queue_num=q + 1)
        for t in range(GATHER_KT):
            kt_tile[NKT_DIRECT + q * GATHER_KT + t] = (wgt, t * NLOC)

    # ---- first 8 k-tiles on the HWDGE rings, arrival-ordered
    eng = {"G": nc.gpsimd, "S": nc.sync, "A": nc.scalar}
    kt0 = 0
    for q, nkt in _SCHED:
        wt = wpool.tile([128, nkt * NLOC], F16, name=f"w{kt0}", tag="w")
        eng[q].dma_start(wt[:], w_d[:, kt0 * NLOC:(kt0 + nkt) * NLOC],
                         single_packet=True)
        for t in range(nkt):
            kt_tile[kt0 + t] = (wt, t * NLOC)
        kt0 += nkt

    # ---- mask pipeline, all in k-major space
    # |x|: clear the f16 sign bit pointwise (int16 view & 0x7fff), two halves
    absx = mk.tile([128, K], F16)
    for h in range(2):
        sl = slice(h * (K // 2), (h + 1) * (K // 2))
        nc.vector.tensor_scalar(absx[:, sl].bitcast(I16),
                                xt_all[:, sl].bitcast(I16),
                                0x7FFF, None, op0=ALU.bitwise_and)

    # part_nT[j, m] = sum_k-in-block-j |x[m, k]| : accumulate 32 PE matmuls
    # whose 0/1 lhsT selectors are shifted windows of ebase
    # (ebase[p, d] = [d == 62 + p//64]  =>  window 62-2kt: [j == 2kt + p//64])
    pnt_ps = psum.tile([64, 128], F32, name="pnt", tag="pnt", bufs=1)
    for kt in range(NKT):
        e_sl = blob[:, _EB0 + 62 - 2 * kt:_EB0 + 126 - 2 * kt]
        nc.tensor.matmul(pnt_ps[:], lhsT=e_sl,
                         rhs=absx[:, kt * 128:(kt + 1) * 128],
                         start=(kt == 0), stop=(kt == NKT - 1))

    # block sums -> means (f16-rounded like jnp.mean): baT[j, b]
    bat_f = mk.tile([64, NBM], F32)
    nc.vector.tensor_reduce(bat_f[:],
                            pnt_ps[:].rearrange("j (b m) -> j b m", m=BLOCK_M),
                            axis=AX.X, op=ALU.add)
    bat16 = mk.tile([64, NBM], F16)
    nc.vector.tensor_scalar_mul(bat16[:], bat_f[:], 1.0 / 2048.0)

    # arow[i, b*64+j] = a[b, j] on 64 partitions: block-diag expand + matmul
    rhs4 = mk.tile([64, NBM * NBK], F16)
    nc.vector.tensor_tensor(
        rhs4[:].rearrange("j (b jj) -> j b jj", b=NBM),
        bat16[:].unsqueeze(-1).broadcast_to((64, NBM, NBK)),
        i64.unsqueeze(1).broadcast_to((64, NBM, NBK)),
        op=ALU.mult)
    ones64 = mk.tile([64, 64], F16)
    nc.vector.memset(ones64[:], 1.0)
    arow_ps = psum.tile([64, NBM * NBK], F32, tag="mkps", bufs=2)
    nc.tensor.matmul(arow_ps[:], lhsT=ones64[:], rhs=rhs4[:], start=True, stop=True)
    arow = mk.tile([64, NBM * NBK], F16)
    nc.vector.tensor_copy(arow[:], arow_ps[:])

    # cnt[i, b] = #{j : a[b, j] > a[b, i]};  keep iff cnt < KEEP
    # (acol[i, b] = a[b, i] is exactly baT16 -- no transpose needed)
    cmp = mk.tile([64, NBM * NBK], F16)
    nc.vector.tensor_tensor(
        cmp[:].rearrange("i (b j) -> i b j", b=NBM),
        arow[:].rearrange("i (b j) -> i b j", b=NBM),
        bat16[:].unsqueeze(-1).broadcast_to((64, NBM, NBK)),
        op=ALU.is_gt)
    cnt = mk.tile([64, NBM], F32)
    nc.vector.tensor_reduce(cnt[:], cmp[:].rearrange("i (b j) -> i b j", b=NBM),
                            axis=AX.X, op=ALU.add)
    keep16 = mk.tile([64, NBM], F16)
    nc.vector.tensor_scalar(keep16[:], cnt[:], float(KEEP), None, op0=ALU.is_lt)

    # keep2[p, kt*4+b] = keep16[2kt + p//64, b]  (kt-major so a 4-ktile xm
    # batch reads a contiguous 16-column slice)
    rhs2 = mk.tile([64, 128], F16)
    nc.vector.tensor_tensor(
        rhs2[:].rearrange("j (kt b) -> j kt b", kt=NKT),
        keep16[:].unsqueeze(1).broadcast_to((64, NKT, NBM)),
        ksel[:].unsqueeze(-1).broadcast_to((64, NKT, NBM)),
        op=ALU.mult)
    ks_ps = psum.tile([128, 128], F32, tag="mkps", bufs=2)
    nc.tensor.matmul(ks_ps[:], lhsT=jh[:], rhs=rhs2[:], start=True, stop=True)
    keep2 = mk.tile([128, 128], F16)
    nc.vector.tensor_copy(keep2[:], ks_ps[:])

    ones = const.tile([1, 128], F16)
    nc.vector.memset(ones[:], 1.0)

    # ---- main GEMM: out[m, n] = sum_kt xm_kt.T @ w_kt + ones.T @ bias ----
    pbanks = [psum.tile([128, 512], F32, name=f"pn{i}", tag=f"pn{i}")
              for i in range(3)]
    # bias as the FIRST accumulation into each bank (start=True) so the
    # banks are complete right when the last k-tile matmul lands
    for nt, (n0, nsz) in enumerate(N_TILES):
        nc.tensor.matmul(pbanks[nt][:, :nsz], lhsT=ones[:],
                         rhs=bias_sb[:, n0:n0 + nsz], start=True, stop=False)

    # masked xT in 4-ktile batches: xm[p, q*32+m] = xt * keep2[p, q]
    xms = []
    for xb in range(NKT // 4):
        xm_b = xmpool.tile([128, 512], F16, name=f"xm{xb}", tag="xm")
        nc.vector.tensor_tensor(
            xm_b[:].rearrange("p (q m) -> p q m", m=BLOCK_M),
            xt_all[:, xb * 512:(xb + 1) * 512].rearrange(
                "p (q m) -> p q m", m=BLOCK_M),
            keep2[:, xb * 16:(xb + 1) * 16].unsqueeze(-1).broadcast_to(
                (128, 16, BLOCK_M)),
            op=ALU.mult)
        xms.append(xm_b)

    for kt in range(NKT):
        lhsT = xms[kt // 4][:, (kt % 4) * 128:(kt % 4 + 1) * 128]
        wt, wbase = kt_tile[kt]
        for nt, (n0, nsz) in enumerate(N_TILES):
            nc.tensor.matmul(pbanks[nt][:, :nsz],
                             lhsT=lhsT,
                             rhs=wt[:, wbase + n0:wbase + n0 + nsz],
                             start=False, stop=(kt == NKT - 1))

    out_sb = opool.tile([128, NLOC], F16)
    out_dma = [nc.sync, nc.scalar, nc.gpsimd]
    pi = 0
    for nt, (n0, nsz) in enumerate(N_TILES):
        for half in range(2):
            h0 = n0 + half * (nsz // 2)
            hsz = nsz // 2 if half == 0 else nsz - nsz // 2
            src = pbanks[nt][:, h0 - n0:h0 - n0 + hsz]
            dst = out_sb[:, h0:h0 + hsz]
            if pi % 2 == 0:
                nc.scalar.activation(dst, src, ACT.Copy)
            else:
                nc.vector.tensor_copy(dst, src)
            out_dma[pi % 3].dma_start(o_d[:, h0:h0 + hsz], dst)
            pi += 1


_CACHE = {}


def _build():
    if "nc" in _CACHE:
        return _CACHE["nc"]
    nc = bacc.Bacc("TRN2", target_bir_lowering=False, debug=False,
                   num_devices=NCORES, num_swdge_queues=4,
                   dynamic_dma_scratch_size=32768)
    xt_d = nc.dram_tensor("xt", (128, K), F16, kind="ExternalInput").ap()
    w_d = nc.dram_tensor("w", (128, NKT_DIRECT * NLOC), F16,
                         kind="ExternalInput").ap()
    wg_d = nc.dram_tensor("wg", (3 * 128, GATHER_KT * NLOC), F16,
                          kind="ExternalInput").ap()
    b_d = nc.dram_tensor("bias", (1, NLOC), F16, kind="ExternalInput").ap()
    blob_d = nc.dram_tensor("blob", (128, _EB0 + _EBW), F16,
                            kind="ExternalInput").ap()
    idx_d = nc.dram_tensor("idx", (128, 24), I16, kind="ExternalInput").ap()
    o_d = nc.dram_tensor("out", (M, NLOC), F16, kind="ExternalOutput").ap()
    with tile.TileContext(nc) as tc:
        with ExitStack() as ctx:
            _program(ctx, tc, [xt_d, w_d, wg_d, b_d, blob_d, idx_d], [o_d])
    nc.compile()
    _CACHE["nc"] = nc
    return nc


def _make_in_maps(x2, weight, bias):
    j_idx = np.arange(64)
    blob_np = np.zeros((128, _EB0 + _EBW), np.float16)
    blob_np[0:64, 0:128] = (
        j_idx[:, None] % 2 == (np.arange(128)[None, :] // 64)).astype(np.float16)
    blob_np[0:64, 128:128 + NKT] = (
        j_idx[:, None] // 2 == np.arange(NKT)[None, :]).astype(np.float16)
    blob_np[0:64, 160:224] = np.eye(64, dtype=np.float16)
    # ebase[p, d] = [d == 62 + p//64]
    blob_np[:, _EB0:] = (
        np.arange(_EBW)[None, :] == 62 + np.arange(128)[:, None] // 64
    ).astype(np.float16)

    # idx[p, q*8+s] = q*128 + s*16 + (p % 16): the [16,8] table for gather q,
    # replicated across all 8 Q7-core partition groups
    idx_np = np.zeros((128, 24), np.int16)
    for q in range(3):
        for p in range(128):
            for sidx in range(8):
                idx_np[p, q * 8 + sidx] = q * 128 + sidx * 16 + (p % 16)

    # x2 [m, kt*128+p] -> xt [p, kt*128+m]
    xt_np = np.ascontiguousarray(
        x2.reshape(M, NKT, 128).transpose(2, 1, 0).reshape(128, K))

    w16 = np.asarray(weight).astype(np.float16, copy=False)
    b16 = np.asarray(bias).astype(np.float16, copy=False)
    in_maps = []
    for c in range(NCORES):
        sl = slice(c * NLOC, (c + 1) * NLOC)
        # [K, NLOC] -> [128, NKT*NLOC]: row p holds W[kt*128+p, :] for all kt
        wp = np.ascontiguousarray(
            w16[:, sl].reshape(NKT, 128, NLOC).transpose(1, 0, 2).reshape(
                128, NKT * NLOC))
        wg = np.ascontiguousarray(np.concatenate(
            [wp[:, (NKT_DIRECT + q * GATHER_KT) * NLOC:
                 (NKT_DIRECT + (q + 1) * GATHER_KT) * NLOC]
             for q in range(3)], axis=0))
        in_maps.append({
            "xt": xt_np,
            "w": np.ascontiguousarray(wp[:, :NKT_DIRECT * NLOC]),
            "wg": wg,
            "bias": np.ascontiguousarray(b16[sl].reshape(1, NLOC)),
            "blob": blob_np,
            "idx": idx_np,
        })
    return in_maps


def kernel(x: np.ndarray, weight: np.ndarray, bias: np.ndarray) -> np.ndarray:
    x = np.asarray(x)
    weight = np.asarray(weight)
    bias = np.asarray(bias)
    bsz, seq, hidden = x.shape
    assert (bsz, seq, hidden) == (M, 1, K) and weight.shape == (K, N)

    x2 = np.ascontiguousarray(x.reshape(M, K).astype(np.float16, copy=False))
    in_maps = _make_in_maps(x2, weight, bias)
    nc = _build()
    res = run_bass_kernel_spmd(nc, in_maps, core_ids=list(range(NCORES)))
    out = np.concatenate([r["out"] for r in res.results], axis=1)
    return out.reshape(M, 1, N).astype(x.dtype, copy=False)


if __name__ == "__main__":
    rng = np.random.default_rng(0)
    x = rng.standard_normal((M, 1, K)).astype(np.float16)
    w = (rng.standard_normal((K, N)) * 0.01).astype(np.float16)
    b = np.zeros((N,), np.float16)
    out = kernel(x, w, b)
    print(out.shape, out.dtype)
